# revision 52
# baseline (speedup 1.0000x reference)
"""Trainium2 Bass kernel for a GPT-style transformer block (B=2, T=2048,
C=1024, 16 heads, MLP 4x), sharded across 8 NeuronCores.

Sharding: attention is HEAD-sharded (core = (batch b=c//4, head group
j=c%4, heads 4j..4j+3)); each core computes q/k/v for its 4 heads over
all 2048 tokens of its batch, so no kv exchange is needed at all. The
head->token re-shard is split into TWO 8-way AllToAlls over token
halves (round-A slot i = tokens [128i,128i+128) of the producer's
batch, round B the same +1024), so dest core i ends up with tokens
[128i,+128) and [1024+128i,+128) of BOTH batches. Round A fires
mid-attention (after subchunk 3) and round B right at its end, so
neither collective is exposed: A lands long before proj needs it and B
hides under MLP-half-A compute.

Schedule (v3): the serial bottleneck is the ACT-engine exp chain (72 x
[128,1024] exps ~ 75us). Everything else is arranged around keeping
that chain fed from ~13us on and keeping the PE p-state high:
- qkv runs as fp8 DoubleRow (gT and qkv weights ship as fp8, weights
  x64, descaled in the DVE drains; q also folds 1/sqrt(d) there) -
  numerically free here and 4x cheaper on PE than bf16.
- Only subchunk 0''s q/k/v are computed up front; the rest stream
  just-in-time as "fillers" between attention items, along with the gT
  quarter loads, the wu prefetch, proj-A, the LN2-A stats, and the a2a
  gathers - all placed by measured deadlines so the exp chain never
  starves for long and the PE stays busy (p-state).
- Attention pipeline: sc(i) | exp(i-1) | av(i-2); the av lag means av
  matmuls never head-of-line block the PE wait queue. PE transposes of
  the attention output borrow the qk PSUM ring; the diagonal (masked)
  kv tiles run mid-subchunk so the boundary normalize chain is never
  behind a mask multiply. PSUM budget: sc 2x2 + av 2 + qk 2 = 8 banks.
- LN2 is folded into the MLP: x1b <- x1b*rstd + (-mu*rstd) runs on the
  Pool+DVE engines off the critical path, so the up matmuls consume
  x1b directly with no normalize pass (stats via ones-matmuls).
- MLP per token half: up-A right at attention end, then proj/stats-B
  (chain-B hides under down-A), down-A, up-B, down-B. up weights stay
  resident in SBUF; down weights stream during MLP-A; proj is fp8
  DoubleRow (x64 weights, descale fused into the residual add).

Numerics: LN1 on host (input-only), LN scale/shift folded into
weights, fp8 a2a payload, fp32 residual stream and softmax
normalization, bf16 MLP (fp8 up measured to breach the 2e-2
tolerance). HW rel err 1.31e-2.
"""
import numpy as np
import ml_dtypes

import concourse.bass as bass
import concourse.mybir as mybir
import concourse.tile as tile
import concourse.bacc as bacc
from concourse.bass_utils import run_bass_kernel_spmd
from concourse.masks import make_identity

BF = ml_dtypes.bfloat16
P = 128
B, T, C, H, D, F = 2, 2048, 1024, 16, 64, 4096
NCT = C // P          # 8   c-tiles
NFT = F // P          # 32  f-tiles
NKT = T // P          # 16  kv tiles per batch
SUB = 256             # q subchunk rows
EPS = 1e-5
f32 = mybir.dt.float32
bf16 = mybir.dt.bfloat16
fp8 = mybir.dt.float8e4
F8 = mybir.dt.np(fp8)
DR = mybir.MatmulPerfMode.DoubleRow
AF = mybir.ActivationFunctionType
ALU = mybir.AluOpType
WS = 64.0         # fp8 weight scale (cleared via the residual fuse)
IWS = 1.0 / WS

_CACHED_NC = None
SKIP_CC = False
DEBUG = False
PROJ_START = 64   # first item index for the proj-A fillers (post a2a#A)


def _build_nc():
    nc = bacc.Bacc("TRN2", target_bir_lowering=False, debug=False)
    d = {}
    for name, shape, dt in [
        ("gT", [C, T], fp8),
        ("WqT", [C, 256], fp8), ("WkT", [C, 256], fp8),
        ("WvT", [C, 256], fp8),
        ("bq", [P, 2], f32), ("bk", [P, 2], f32), ("brep", [P, 256], bf16),
        ("maskA", [P, 1024], bf16), ("maskB", [P, 1024], bf16),
        ("WpT", [C, C], fp8), ("xbT", [C, 512], f32),
        ("WupT", [NFT, P, NCT, P], bf16), ("bup", [P, NFT], f32),
        ("wru", [P, NFT], f32),
        ("WdownT", [NFT, P, C], bf16), ("bdown", [P, NCT], f32),
    ]:
        d[name] = nc.dram_tensor(name, shape, dt, kind="ExternalInput").ap()
    d["OUT"] = nc.dram_tensor("OUT", [C, 512], f32, kind="ExternalOutput").ap()
    if DEBUG:
        for name, shape, dt in [("dbg_sc", [P, 2, 1024], f32),
                                ("dbg_ex", [P, 2, 1024], bf16),
                                ("dbg_den", [P, 8], f32),
                                ("dbg_qT", [P, 2, 512], bf16),
                                ("dbg_kT", [P, 2, 512], bf16)]:
            d[name] = nc.dram_tensor(name, shape, dt,
                                     kind="ExternalOutput").ap()

    with tile.TileContext(nc) as tc:
        _emit(nc, tc, d)
    nc.compile()
    return nc


def _emit(nc, tc, d):
    from contextlib import ExitStack

    with ExitStack() as ctx:
        cpool = ctx.enter_context(tc.tile_pool(name="cpool", bufs=1))
        prepool = ctx.enter_context(tc.tile_pool(name="prepool", bufs=1))
        dramp = ctx.enter_context(tc.tile_pool(name="dramp", bufs=1,
                                               space="DRAM"))

        # persistent small tiles
        bq = cpool.tile([P, 2], f32, name="bq")
        bk = cpool.tile([P, 2], f32, name="bk")
        brep = cpool.tile([P, 256], bf16, name="brep")
        bup = cpool.tile([P, NFT], f32, name="bup")
        wru = cpool.tile([P, NFT], f32, name="wru")
        bdown = cpool.tile([P, NCT], f32, name="bdown")
        epsT = cpool.tile([P, 1], f32, name="epsT")
        onesb = cpool.tile([P, P], bf16, name="onesb")
        ident = cpool.tile([P, P], bf16, name="ident")
        xbT = cpool.tile([P, NCT, 512], f32, name="xbT")
        # post-a2a attention output, all channels x my 256 tokens, per half
        attnT_A = cpool.tile([P, NCT, 256], fp8, name="attnT_A")
        attnT_B = cpool.tile([P, NCT, 256], fp8, name="attnT_B")
        # proj+residual stream and its bf16/square copies (LN2 stats)
        x1T = cpool.tile([P, NCT, 512], f32, name="x1T")
        x1b = cpool.tile([P, NCT, 256], bf16, name="x1b")
        sqb = cpool.tile([P, NCT, 256], bf16, name="sqb")
        # per-half LN2 row vectors (partition-replicated): 1/std and -mu/std
        rstdv = [cpool.tile([P, 256], f32, name=f"rstd{h}") for h in range(2)]
        msneg = [cpool.tile([P, 256], f32, name=f"msneg{h}") for h in range(2)]
        nc.vector.memset(epsT[:], EPS)
        nc.vector.memset(onesb[:], 1.0)
        make_identity(nc, ident[:])
        for t, key in [(bq, "bq"), (bk, "bk"), (brep, "brep"), (bup, "bup"),
                       (wru, "wru"), (bdown, "bdown")]:
            nc.gpsimd.dma_start(t[:], d[key])

        # resident weights: proj (fp8) + MLP up (both halves consume it)
        wpT = prepool.tile([P, NCT, C], fp8, name="wpT")
        wu = prepool.tile([P, NFT, NCT, P], bf16, name="wu")

        # a2a DRAM staging, split into two token-half rounds over all 8
        # cores: round-A slot i = tokens [128i, 128i+128) of my batch
        # (ready after subchunk i//2 <= 3), round-B slot i = the same
        # +1024. Dest i thus gets tokens [128i,+128) of BOTH batches per
        # round (its 256 MLP token-columns for that half).
        a2aA_in = dramp.tile([8, 256, 128], fp8, name="a2aA_in")
        a2aA_out = dramp.tile([8, 256, 128], fp8, name="a2aA_out")
        a2aB_in = dramp.tile([8, 256, 128], fp8, name="a2aB_in")
        a2aB_out = dramp.tile([8, 256, 128], fp8, name="a2aB_out")

        def run_a2a(src, dst):
            if SKIP_CC:
                nc.gpsimd.dma_start(dst[:], src[:])
            else:
                nc.gpsimd.collective_compute(
                    "AllToAll", ALU.bypass,
                    ins=[src[:]], outs=[dst[:]],
                    replica_groups=[[0, 1, 2, 3, 4, 5, 6, 7]])

        def emit_gather(out_t, attnT_h, b2, queue):
            # all 4 source slots of one batch in a single DMA: (j, dt)
            # merge to a uniform-stride dim
            queue.dma_start(
                attnT_h[:, :, b2 * P:(b2 + 1) * P],
                out_t[4 * b2:4 * b2 + 4].rearrange(
                    "j (dt p) t -> p (j dt) t", p=P, dt=2))

        # ============ fused QKV + attention + proj-A phase ============
        with ExitStack() as actx:
            bpool = actx.enter_context(tc.tile_pool(name="bpool", bufs=1))
            qT = bpool.tile([P, 2, T], bf16, name="qT")
            kT = bpool.tile([P, 2, T], bf16, name="kT")
            v4 = bpool.tile([P, 4, NKT, 65], bf16, name="v4")
            attn_cT = bpool.tile([P, 2, T], fp8, name="attn_cT")
            maskA = bpool.tile([P, 1024], bf16, name="maskA")
            maskB = bpool.tile([P, 1024], bf16, name="maskB")
            nc.vector.memset(v4[:, :, :, 64:65], 1.0)

            qkvp = actx.enter_context(tc.tile_pool(name="qkvp", bufs=1))
            wqT = qkvp.tile([P, NCT, 256], fp8, name="wqT")
            wkT = qkvp.tile([P, NCT, 256], fp8, name="wkT")
            wvT = qkvp.tile([P, NCT, 256], fp8, name="wvT")
            # gT streamed per 512-token quarter through a 3-deep ring
            # (quarter t serves exactly subchunks 2t, 2t+1 for q, k, v)
            gqp = actx.enter_context(tc.tile_pool(name="gqp", bufs=3))

            expp = actx.enter_context(tc.tile_pool(name="expp", bufs=5))
            nrmp = actx.enter_context(tc.tile_pool(name="nrmp", bufs=3))
            asbp = actx.enter_context(tc.tile_pool(name="asbp", bufs=4))
            scps = actx.enter_context(
                tc.tile_pool(name="scps", bufs=2, space="PSUM"))
            avps = actx.enter_context(
                tc.tile_pool(name="avps", bufs=1, space="PSUM"))
            qkps = actx.enter_context(
                tc.tile_pool(name="qkps", bufs=2, space="PSUM"))

            gsrc = d["gT"].rearrange("(ct p) t -> p ct t", p=P)
            gq_t = {}

            def emit_gq(tq, split=False):
                gq = gqp.tile([P, NCT, 512], fp8, name="gq", tag="gq")
                if split:
                    for ct in range(NCT):
                        nc.sync.dma_start(
                            gq[:, ct, :], gsrc[:, ct, tq * 512:(tq + 1) * 512])
                else:
                    nc.sync.dma_start(gq[:],
                                      gsrc[:, :, tq * 512:(tq + 1) * 512])
                gq_t[tq] = gq

            # DMA order on the sync queue = DMA_ENGINES service order
            nc.sync.dma_start(
                wkT[:], d["WkT"].rearrange("(ct p) o -> p ct o", p=P))
            emit_gq(0, split=True)
            nc.sync.dma_start(
                wqT[:], d["WqT"].rearrange("(ct p) o -> p ct o", p=P))
            nc.sync.dma_start(
                wvT[:], d["WvT"].rearrange("(ct p) o -> p ct o", p=P))
            emit_gq(1)
            nc.gpsimd.dma_start(maskA[:], d["maskA"])
            nc.gpsimd.dma_start(maskB[:], d["maskB"])
            nc.sync.dma_start(
                wpT[:], d["WpT"].rearrange("(ct p) o -> p ct o", p=P))
            nc.sync.dma_start(
                xbT[:], d["xbT"].rearrange("(ct p) t -> p ct t", p=P))

            def emit_qk(w, bias, dst, tq, hp):
                # fp8 DoubleRow over c-tile pairs; the x64 weight scale is
                # cleared in the drain
                gq = gq_t[tq]
                pq = qkps.tile([P, 512], f32, name="pq", tag="qk")
                for c2 in range(4):
                    nc.tensor.matmul(
                        pq[:], w[:, 2 * c2:2 * c2 + 2, hp * P:(hp + 1) * P],
                        gq[:, 2 * c2:2 * c2 + 2, :],
                        start=(c2 == 0), stop=(c2 == 3), perf_mode=DR)
                nc.vector.tensor_scalar(
                    dst[:, hp, tq * 512:(tq + 1) * 512], pq[:], IWS,
                    bias[:, hp:hp + 1], ALU.mult, ALU.add)

            def emit_v(tt):
                gq = gq_t[tt // 4]
                off = (tt % 4) * P
                pv = qkps.tile([P, 256], f32, name="pv", tag="qk")
                for c2 in range(4):
                    nc.tensor.matmul(
                        pv[:], gq[:, 2 * c2:2 * c2 + 2, off:off + P],
                        wvT[:, 2 * c2:2 * c2 + 2, :],
                        start=(c2 == 0), stop=(c2 == 3), perf_mode=DR)
                nc.vector.scalar_tensor_tensor(
                    v4[:, :, tt, 0:64],
                    pv[:].rearrange("p (h e) -> p h e", e=64), IWS,
                    brep[:].rearrange("p (h e) -> p h e", e=64),
                    ALU.mult, ALU.add)

            def emit_proj(half, attnT_h, psum_pool, ot):
                sl = slice(half * 256, (half + 1) * 256)
                pp = psum_pool.tile([P, 256], f32, name="pp", tag="qk")
                for dk in range(4):
                    nc.tensor.matmul(
                        pp[:], wpT[:, 2 * dk:2 * dk + 2, ot * P:(ot + 1) * P],
                        attnT_h[:, 2 * dk:2 * dk + 2, :],
                        start=(dk == 0), stop=(dk == 3), perf_mode=DR)
                nc.vector.scalar_tensor_tensor(
                    x1T[:, ot, sl], pp[:], IWS, xbT[:, ot, sl],
                    ALU.mult, ALU.add)
                nc.vector.tensor_copy(x1b[:, ot, :], x1T[:, ot, sl])
                nc.vector.tensor_mul(sqb[:, ot, :], x1b[:, ot, :],
                                     x1b[:, ot, :])

            stats_t = {}

            def emit_stats(psum_pool, which):
                src = x1b if which == 0 else sqb
                ps = psum_pool.tile([P, 256], f32, name="pstat", tag="qk")
                for ct in range(NCT):
                    nc.tensor.matmul(ps[:], onesb[:], src[:, ct, :],
                                     start=(ct == 0), stop=(ct == NCT - 1))
                stats_t[which] = ps

            def emit_chain(half, scratch_pool):
                """LN2 stats -> rstd, -mu/std (all DVE except the sqrt)."""
                psmu = stats_t.pop(0)
                pssq = stats_t.pop(1)
                mu = scratch_pool.tile([P, 256], f32, name="mu", tag="mu", bufs=1)
                e2 = scratch_pool.tile([P, 256], f32, name="e2", tag="e2", bufs=1)
                nc.vector.tensor_scalar_mul(mu[:], psmu[:], 1.0 / C)
                nc.vector.tensor_scalar_mul(e2[:], pssq[:], 1.0 / C)
                musq = scratch_pool.tile([P, 256], f32, name="musq",
                                         tag="musq", bufs=1)
                nc.vector.tensor_mul(musq[:], mu[:], mu[:])
                nc.vector.tensor_sub(e2[:], e2[:], musq[:])
                std = scratch_pool.tile([P, 256], f32, name="std", tag="std", bufs=1)
                nc.scalar.activation(std[:], e2[:], AF.Sqrt, bias=epsT[:])
                nc.vector.reciprocal(rstdv[half][:], std[:])
                nc.vector.scalar_tensor_tensor(
                    msneg[half][:], mu[:], -1.0, rstdv[half][:],
                    ALU.mult, ALU.mult)

            def emit_g2(half, cts, dve=False):
                # x1b <- x1b*rstd + (-mu*rstd), in place, on the Pool (or
                # DVE) engine, off the up-matmul critical path
                eng = nc.vector if dve else nc.gpsimd
                for ct in cts:
                    eng.tensor_mul(x1b[:, ct, :], x1b[:, ct, :],
                                   rstdv[half][:])
                    eng.tensor_add(x1b[:, ct, :], x1b[:, ct, :],
                                   msneg[half][:])

            # warm up the PE p-state during the initial DMA wait:
            # dependency-free transposes keep the engine continuously busy
            # from ~0.5us so the first k/q/v matmuls run at full clock
            # instead of the post-idle 0.65/1.2 GHz p-states
            warm = qkps.tile([P, P], f32, name="warm", tag="qk")
            for _ in range(55):
                nc.tensor.matmul(warm[:], ident[:], onesb[:],
                                 start=False, stop=False,
                                 skip_group_check=True)

            # pre-attention: only what subchunk 0 needs (all from gT
            # quarter 0) so the ACT exp chain starts ASAP
            emit_qk(wkT, bk, kT, 0, 0)
            emit_qk(wkT, bk, kT, 0, 1)
            emit_qk(wqT, bq, qT, 0, 0)
            emit_qk(wqT, bq, qT, 0, 1)
            emit_v(0)
            emit_v(1)

            # the rest of qkv, the gT quarter loads, the proj-A/LN2-A
            # stats matmuls and the a2a#A gathers are emitted just-in-time
            # inside the item loop, filling PE stalls while ACT runs the
            # exp chain — this also keeps the PE p-state high
            fillers = {}  # item index -> [closures]

            def add_filler(i, fn):
                fillers.setdefault(max(i, 0), []).append(fn)

            def qk_f(w, b_, dst, tq, hp):
                return lambda: emit_qk(w, b_, dst, tq, hp)

            add_filler(1, lambda: emit_v(2))
            add_filler(2, lambda: emit_v(3))
            add_filler(3, qk_f(wkT, bk, kT, 1, 0))
            add_filler(4, qk_f(wkT, bk, kT, 1, 1))
            add_filler(4, qk_f(wqT, bq, qT, 1, 0))
            add_filler(5, qk_f(wqT, bq, qT, 1, 1))
            add_filler(7, lambda: emit_v(4))
            add_filler(8, lambda: emit_v(5))
            add_filler(10, lambda: emit_gq(2))
            add_filler(13, lambda: emit_v(6))
            add_filler(14, lambda: emit_v(7))
            add_filler(15, qk_f(wqT, bq, qT, 2, 0))
            add_filler(16, qk_f(wqT, bq, qT, 2, 1))
            add_filler(17, qk_f(wkT, bk, kT, 2, 0))
            add_filler(18, qk_f(wkT, bk, kT, 2, 1))
            add_filler(21, lambda: emit_v(8))
            add_filler(22, lambda: emit_v(9))
            add_filler(31, lambda: emit_gq(3))
            add_filler(31, lambda: emit_v(10))
            add_filler(32, lambda: emit_v(11))
            add_filler(35, qk_f(wqT, bq, qT, 3, 0))
            add_filler(36, qk_f(wqT, bq, qT, 3, 1))
            add_filler(38, qk_f(wkT, bk, kT, 3, 0))
            add_filler(39, qk_f(wkT, bk, kT, 3, 1))
            add_filler(43, lambda: emit_v(12))
            add_filler(44, lambda: emit_v(13))
            add_filler(57, lambda: emit_v(14))
            add_filler(58, lambda: emit_v(15))
            for ot in range(NFT):
                add_filler(42 + ot,
                           lambda ot=ot: nc.sync.dma_start(wu[:, ot],
                                                           d["WupT"][ot]))
            # a2a#A gathers once the collective is surely done (sync
            # queue: HWDGE descriptor gen is much faster than SWDGE)
            for b2 in range(2):
                add_filler(PROJ_START - 3 + b2,
                           lambda b2=b2:
                           emit_gather(a2aA_out, attnT_A, b2, nc.sync))
            # proj-A + LN2-A stats into the qk PSUM ring
            for ot in range(NCT):
                add_filler(PROJ_START + ot,
                           lambda ot=ot: emit_proj(0, attnT_A, qkps, ot))
            add_filler(PROJ_START + 8, lambda: emit_stats(qkps, 0))
            add_filler(PROJ_START + 9, lambda: emit_stats(qkps, 1))
            add_filler(PROJ_START + 12, lambda: emit_chain(0, nrmp))
            add_filler(PROJ_START + 13, lambda: emit_g2(0, range(0, 4)))
            add_filler(PROJ_START + 14, lambda: emit_g2(0, range(4, NCT),
                                                        dve=True))

            # software-pipelined attention: emit sc(i+1) before exp/av(i);
            # the epilogue's PE transposes are deferred one stage further
            items = [(s, kt) for s in range(8)
                     for kt in ([0, 2 * s, 2 * s + 1] + list(range(1, 2 * s))
                                if s else [0, 1])]
            avq_s, sc_t = {}, {}

            def alloc_av(s):
                # NB: start=True on a PSUM matmul zeroes the whole bank,
                # wiping the other head regions sharing it — so the
                # accumulators must be memset and accumulated start=False
                avA = avps.tile([P, 4, 65], f32, name="avA", tag="avA")
                avB = avps.tile([P, 4, 65], f32, name="avB", tag="avB")
                nc.vector.memset(avA[:], 0.0)
                nc.vector.memset(avB[:], 0.0)
                avq_s[s] = (avA, avB)

            def emit_sc(i):
                s, kt = items[i]
                sc = scps.tile([P, 1024], f32, name="sc", tag="sc")
                sc_t[i] = sc
                for h in range(4):
                    hb = (h % 2) * 64
                    colo = (h % 2) * 512 + (h // 2) * 256
                    nc.tensor.matmul(
                        sc[:, colo:colo + 256],
                        kT[hb:hb + 64, h // 2, kt * P:(kt + 1) * P],
                        qT[hb:hb + 64, h // 2, s * SUB:(s + 1) * SUB],
                        start=True, stop=True)

            ex_t = {}

            def emit_exp(i):
                s, kt = items[i]
                ex = expp.tile([P, 1024], bf16, name="ex", tag="ex")
                nc.scalar.activation(ex[:], sc_t.pop(i)[:], AF.Exp)
                if kt == 2 * s:
                    nc.vector.tensor_mul(ex[:], ex[:], maskA[:])
                elif kt == 2 * s + 1:
                    nc.vector.tensor_mul(ex[:], ex[:], maskB[:])
                ex_t[i] = ex

            def emit_rest(i):
                """av matmuls; at subchunk end also the DVE normalize.
                Returns (s, asb) at a subchunk boundary, else None."""
                s, kt = items[i]
                nkv = 2 * s + 2
                avq = avq_s[s]
                pos = i - (s * s + s)        # position within the subchunk
                ex = ex_t.pop(i)
                for qt in range(2):
                    for h in range(4):
                        colo = (h % 2) * 512 + (h // 2) * 256
                        nc.tensor.matmul(
                            avq[qt][:, h, :],
                            ex[:, colo + qt * P:colo + (qt + 1) * P],
                            v4[:, h, kt, :],
                            start=False, stop=(pos == nkv - 1),
                            skip_group_check=True)
                if pos != nkv - 1:
                    return None
                avq = avq_s.pop(s)
                den = nrmp.tile([P, 8], f32, name="den", tag="den")
                rden = nrmp.tile([P, 8], f32, name="rden", tag="rden")
                for qt in range(2):
                    nc.vector.tensor_copy(
                        den[:, qt * 4:(qt + 1) * 4],
                        avq[qt][:, :, 64:65].rearrange("p h e -> p (h e)"))
                if DEBUG and s == 0:
                    nc.sync.dma_start(d["dbg_den"], den[:])
                    nc.sync.dma_start(d["dbg_qT"], qT[:, :, 0:512])
                    nc.sync.dma_start(d["dbg_kT"], kT[:, :, 0:512])
                nc.vector.reciprocal(rden[:], den[:])
                asb = asbp.tile([P, 2, 256], bf16, name="asb", tag="asb")
                for qt in range(2):
                    rq = rden[:, qt * 4:(qt + 1) * 4].rearrange(
                        "p (h u) -> p h u", u=1).broadcast_to([P, 4, 64])
                    nc.vector.tensor_mul(
                        asb[:, qt, :].rearrange("p (h e) -> p h e", e=64),
                        avq[qt][:, :, 0:64], rq)
                if s + 1 < 8:
                    alloc_av(s + 1)   # after the normalize reads (bufs=1)
                return (s, asb)

            def emit_tps(s, asb):
                # transposes borrow the sc PSUM ring (tag reuse) — by the
                # time asb is ready, the ring's previous exps are long
                # done. attn_cT copies go to the Pool engine so they never
                # clog the DVE wait queue; staging writes ride the sync
                # queue (HWDGE).
                for qt in range(2):
                    for dt in range(2):
                        tps = qkps.tile([P, P], bf16, name="tps", tag="qk")
                        nc.tensor.transpose(tps[:],
                                            asb[:, qt, dt * P:(dt + 1) * P],
                                            ident[:])
                        nc.vector.tensor_copy(
                            attn_cT[:, dt, s * SUB + qt * P:
                                    s * SUB + (qt + 1) * P],
                            tps[:])
                half, sl2 = divmod(s, 4)
                a2a_in = a2aA_in if half == 0 else a2aB_in
                for q2 in range(2):
                    nc.sync.dma_start(
                        a2a_in[2 * sl2 + q2].rearrange(
                            "(dt p) t -> p dt t", p=P),
                        attn_cT[:, :, s * SUB + q2 * P:s * SUB + (q2 + 1) * P])
                if s == 3:
                    run_a2a(a2aA_in, a2aA_out)
                elif s == 7:
                    run_a2a(a2aB_in, a2aB_out)

            alloc_av(0)
            # tps(s) is deferred TPS_LAG items past the subchunk boundary
            # so the PE queue never head-of-line blocks on the boundary's
            # DVE normalize chain
            TPS_LAG = 3
            pend_q = []
            for i in range(len(items) + 3 + TPS_LAG):
                if i < len(items):
                    emit_sc(i)
                for fn in fillers.pop(i, ()):
                    fn()
                while pend_q and i - pend_q[0][0] >= TPS_LAG:
                    _, s_, asb_ = pend_q.pop(0)
                    emit_tps(s_, asb_)
                if 1 <= i <= len(items):
                    emit_exp(i - 1)
                if 2 <= i <= len(items) + 1:
                    pend = emit_rest(i - 2)
                    if pend is not None:
                        pend_q.append((i, *pend))
            for i, fns in sorted(fillers.items()):
                for fn in fns:
                    fn()

        # attention + qkv pools freed (SBUF and all 8 PSUM banks)

        # ============ MLP on my 512 tokens, per token half ============
        # LN2 normalization is folded into up: with rowsum_o = sum_c W[o,c],
        # up_o(t) = rstd(t)*(W @ x1b)_o(t) + (-mu(t)*rstd(t))*rowsum_o,
        # applied on DVE before the gelu — no separate normalize pass.
        with tc.tile_pool(name="mlpp", bufs=1) as mlpp, \
             tc.tile_pool(name="pps", bufs=2, space="PSUM") as pps, \
             tc.tile_pool(name="lnps", bufs=2, space="PSUM") as lnps, \
             tc.tile_pool(name="upps", bufs=2, space="PSUM") as upps, \
             tc.tile_pool(name="dps", bufs=2, space="PSUM") as dps:
            wd = mlpp.tile([P, NFT, C], bf16, name="wd")
            for cf in range(NFT):
                nc.sync.dma_start(wd[:, cf, :], d["WdownT"][cf])
            hT = mlpp.tile([P, NFT, 256], bf16, name="hT")
            outdst = d["OUT"].rearrange("(ot p) t -> ot p t", p=P)

            def emit_up(half):
                for ot in range(NFT):
                    pu = upps.tile([P, 256], f32, name="pu", tag="pu")
                    for ct in range(NCT):
                        nc.tensor.matmul(pu[:], wu[:, ot, ct, :],
                                         x1b[:, ct, :],
                                         start=(ct == 0),
                                         stop=(ct == NCT - 1))
                    nc.scalar.activation(hT[:, ot, :], pu[:], AF.Gelu,
                                         bias=bup[:, ot:ot + 1])

            def emit_down(half):
                sl = slice(half * 256, (half + 1) * 256)
                for ot in range(NCT):
                    pd = dps.tile([P, 256], f32, name="pd", tag="pd")
                    for cf in range(NFT):
                        nc.tensor.matmul(
                            pd[:], wd[:, cf, ot * P:(ot + 1) * P],
                            hT[:, cf, :],
                            start=(cf == 0), stop=(cf == NFT - 1))
                    ou = mlpp.tile([P, 256], f32, name="ou", tag="ou",
                                   bufs=2)
                    nc.vector.scalar_tensor_tensor(
                        ou[:], pd[:], bdown[:, ot:ot + 1], x1T[:, ot, sl],
                        ALU.add, ALU.add)
                    nc.sync.dma_start(outdst[ot][:, sl], ou[:])

            # half A: x1b/stats/chain already done during attention
            emit_up(0)
            # half B proj + stats, then chain + down-A (chain hides under
            # the down matmuls), then up-B, down-B
            for b2 in range(2):
                emit_gather(a2aB_out, attnT_B, b2, nc.sync)
            for ot in range(NCT):
                emit_proj(1, attnT_B, pps, ot)
            emit_stats(lnps, 0)
            emit_stats(lnps, 1)
            emit_chain(1, mlpp)
            emit_g2(1, range(NCT))
            emit_down(0)
            emit_up(1)
            emit_down(1)


def _prep_inputs(x, ln1_w, ln1_b, c_attn_w, c_attn_b, c_proj_w, c_proj_b,
                 ln2_w, ln2_b, up_w, up_b, down_w, down_b):
    """Host-side preprocessing -> list of 8 per-core input dicts."""
    x = np.asarray(x, np.float32)
    f64 = np.float64
    # LN1 on host (pure function of the input)
    mu = x.mean(-1, keepdims=True, dtype=f64)
    var = np.asarray(x, f64).var(-1, keepdims=True)
    g = ((x - mu) / np.sqrt(var + EPS)).astype(np.float32)     # [B, T, C]

    ln1_w = np.asarray(ln1_w, np.float32); ln1_b = np.asarray(ln1_b, np.float32)
    ln2_w = np.asarray(ln2_w, np.float32); ln2_b = np.asarray(ln2_b, np.float32)
    c_attn_w = np.asarray(c_attn_w, np.float32)
    c_attn_b = np.asarray(c_attn_b, np.float32)
    c_proj_w = np.asarray(c_proj_w, np.float32)
    c_proj_b = np.asarray(c_proj_b, np.float32)
    up_w = np.asarray(up_w, np.float32); up_b = np.asarray(up_b, np.float32)
    down_w = np.asarray(down_w, np.float32)
    down_b = np.asarray(down_b, np.float32)

    Wa = c_attn_w * ln1_w[None, :]                  # fold LN1 scale
    ba = c_attn_b + c_attn_w @ ln1_b                # fold LN1 shift
    Wq, Wk, Wv = Wa[:C], Wa[C:2 * C], Wa[2 * C:]
    bqv, bkv, bvv = ba[:C], ba[C:2 * C], ba[2 * C:]
    s = 1.0 / np.sqrt(D)
    # qkv weights ship as fp8 scaled x(64*8) for q (the attention scale
    # rides in the drain descale with WS) and x64 for k/v
    Wq = Wq * (s * WS); bqv = bqv * s
    Wk = Wk * WS
    Wv = Wv * WS

    Wup = up_w * ln2_w[None, :]
    bupv = up_b + up_w @ ln2_b

    # causal masks for kv tile vs 256-row q subchunk (diagonal tiles), same
    # on every core; tiled x4 across the 4 packed heads
    tk = np.arange(P)[:, None]
    tq = np.arange(SUB)[None, :]
    mA = np.tile((tk <= tq).astype(np.float32), (1, 4))
    mB = np.tile((tk + P <= tq).astype(np.float32), (1, 4))

    shared = {
        "WpT": np.ascontiguousarray(c_proj_w.T * WS).astype(F8),
        "WupT": np.ascontiguousarray(
            Wup.reshape(NFT, P, NCT, P).transpose(0, 3, 2, 1)).astype(BF),
        "bup": np.ascontiguousarray(bupv.reshape(NFT, P).T).astype(np.float32),
        "wru": np.ascontiguousarray(
            Wup.astype(BF).astype(np.float32).sum(1).reshape(NFT, P).T
        ).astype(np.float32),
        "WdownT": np.ascontiguousarray(
            down_w.T.reshape(NFT, P, C)).astype(BF),
        "bdown": np.ascontiguousarray(
            down_b.reshape(NCT, P).T).astype(np.float32),
        "maskA": mA.astype(BF), "maskB": mB.astype(BF),
    }

    xb = x + c_proj_b[None, None, :]                # fold proj bias in residual
    gT_b = [np.ascontiguousarray(g[b].T).astype(F8) for b in range(B)]
    xbT_b = [np.ascontiguousarray(xb[b].T).astype(np.float32) for b in range(B)]

    # head-pair reorder for q/k: col (hp*128 + (h%2)*64 + d) = head 4j+2hp+(h%2)
    def qk_slice(W, bias, j):
        rows = W.reshape(H, D, C)[4 * j:4 * j + 4]      # [4, 64, C]
        b4 = bias.reshape(H, D)[4 * j:4 * j + 4]
        order = [0, 1, 2, 3]                            # hp0: h0,h1; hp1: h2,h3
        rows = rows[order].reshape(2, 2 * D, C)         # [hp, 128, C]
        b4 = b4[order].reshape(2, 2 * D)
        WT = np.ascontiguousarray(rows.reshape(256, C).T).astype(F8)
        bT = np.ascontiguousarray(b4.reshape(2, P).T).astype(np.float32)
        return WT, bT

    in_maps = []
    for core in range(8):
        b, j = core // 4, core % 4
        m = dict(shared)
        m["gT"] = gT_b[b]
        WqTs, bqs = qk_slice(Wq, bqv, j)
        WkTs, bks = qk_slice(Wk, bkv, j)
        m["WqT"] = WqTs; m["bq"] = bqs
        m["WkT"] = WkTs; m["bk"] = bks
        m["WvT"] = np.ascontiguousarray(
            Wv[256 * j:256 * (j + 1)].T).astype(F8)
        m["brep"] = np.broadcast_to(
            bvv[256 * j:256 * (j + 1)].astype(BF), (P, 256)).copy()
        # token ownership: half A = tokens [128c, 128c+128) of BOTH
        # batches, half B = the same +1024 (c = core index)
        c0 = 128 * core
        m["xbT"] = np.concatenate(
            [xbT_b[0][:, c0:c0 + 128], xbT_b[1][:, c0:c0 + 128],
             xbT_b[0][:, 1024 + c0:1024 + c0 + 128],
             xbT_b[1][:, 1024 + c0:1024 + c0 + 128]], axis=1)
        in_maps.append(m)
    return in_maps


def kernel(**inputs):
    global _CACHED_NC
    if _CACHED_NC is None:
        _CACHED_NC = _build_nc()
    nc = _CACHED_NC
    in_maps = _prep_inputs(**inputs)
    try:
        res = run_bass_kernel_spmd(nc, in_maps, list(range(8)))
    except Exception:
        # one retry: transient NRT device faults are recoverable on re-run
        res = run_bass_kernel_spmd(nc, in_maps, list(range(8)))
    out = np.empty((B, T, C), np.float32)
    for core in range(8):
        c0 = 128 * core
        o = res.results[core]["OUT"]                # [C, 512]
        out[0, c0:c0 + 128, :] = o[:, 0:128].T
        out[1, c0:c0 + 128, :] = o[:, 128:256].T
        out[0, 1024 + c0:1024 + c0 + 128, :] = o[:, 256:384].T
        out[1, 1024 + c0:1024 + c0 + 128, :] = o[:, 384:512].T
    return out
